# revision 1
# baseline (speedup 1.0000x reference)
"""Causal varlen GQA flash attention (prefill) on 8 TRN2 NeuronCores.

Problem shape (hardcoded): B=8 sequences x S=1024 tokens, 32 q heads /
8 kv heads (GQA group 4), head_dim 128, fp32 in/out, causal.

Sharding: tensor-parallel over kv heads. Core c owns kv head c and its
4 query heads: q cols [512c, 512c+512), k/v cols [128c, 128c+128),
output cols [512c, 512c+512). No collectives; host concatenates.

Per-core kernel (all matmuls bf16, fp32 PSUM accumulation):
  S^T[k,q] = (K^T block).T @ Q^T          PE, causally trimmed N
  P^T      = exp(scale * S^T)             ScalarE, PSUM->SBUF bf16
  diag blocks masked with triangular mask ScalarE/DVE
  O[q,d]  += (P^T block).T @ V block      PE (P^T stationary, V moving)
  den[q]  += (P^T block).T @ ones         PE (same stationary, N=1)
  out      = O * (1/den)                  DVE reciprocal + scale
"""

import numpy as np
import ml_dtypes
from contextlib import ExitStack

import concourse.bacc as bacc
import concourse.bass as bass
import concourse.mybir as mybir
import concourse.tile as tile
from concourse.bass_utils import run_bass_kernel_spmd

B = 8
S = 1024
D = 128
GH = 4            # q heads per core
NT = S // 128     # 128-token tiles per sequence
NC = 8            # cores
SCALE = 1.0 / float(np.sqrt(D))
F32 = mybir.dt.float32
BF16 = mybir.dt.bfloat16

_CACHE: dict = {}


def _build_nc(b_count=B, h_count=GH):
    nc = bacc.Bacc("TRN2", target_bir_lowering=False, debug=False)
    q_d = nc.dram_tensor("q", [B * S, GH * D], F32, kind="ExternalInput")
    k_d = nc.dram_tensor("k", [B * S, D], F32, kind="ExternalInput")
    v_d = nc.dram_tensor("v", [B * S, D], F32, kind="ExternalInput")
    m_d = nc.dram_tensor("trimask", [128, 128], BF16, kind="ExternalInput")
    one_d = nc.dram_tensor("onecol", [128, 1], BF16, kind="ExternalInput")
    o_d = nc.dram_tensor("o", [B * S, GH * D], F32, kind="ExternalOutput")

    with tile.TileContext(nc) as tc, ExitStack() as ctx:
        cpool = ctx.enter_context(tc.tile_pool(name="const", bufs=1))
        kvpool = ctx.enter_context(tc.tile_pool(name="kv", bufs=2))
        qpool = ctx.enter_context(tc.tile_pool(name="qp", bufs=2))
        ppool = ctx.enter_context(tc.tile_pool(name="pp", bufs=2))
        opool = ctx.enter_context(tc.tile_pool(name="op", bufs=2))
        rpool = ctx.enter_context(tc.tile_pool(name="rp", bufs=2))
        psS = ctx.enter_context(tc.tile_pool(name="psS", bufs=3, space="PSUM"))
        psO = ctx.enter_context(tc.tile_pool(name="psO", bufs=2, space="PSUM"))
        psD = ctx.enter_context(tc.tile_pool(name="psD", bufs=2, space="PSUM"))

        mask_sb = cpool.tile([128, 128], BF16, name="mask_sb")
        nc.sync.dma_start(out=mask_sb[:], in_=m_d[:])
        ones_sb = cpool.tile([128, 1], BF16, name="ones_sb")
        nc.sync.dma_start(out=ones_sb[:], in_=one_d[:])

        for b in range(b_count):
            rows = slice(b * S, (b + 1) * S)
            kn = kvpool.tile([128, NT, 128], BF16, tag="kn", name="kn")
            nc.gpsimd.dma_start(
                out=kn[:], in_=k_d[rows, :].rearrange("(t p) d -> p t d", p=128)
            )
            vn = kvpool.tile([128, NT, 128], BF16, tag="vn", name="vn")
            nc.gpsimd.dma_start(
                out=vn[:], in_=v_d[rows, :].rearrange("(t p) d -> p t d", p=128)
            )
            kt = kvpool.tile([128, S], BF16, tag="kt", name="kt")
            for t in range(NT):
                nc.sync.dma_start_transpose(
                    out=kt[:, t * 128 : (t + 1) * 128], in_=kn[:, t, :]
                )
            for h in range(h_count):
                qn = qpool.tile([128, NT, 128], BF16, tag="qn", name="qn")
                nc.gpsimd.dma_start(
                    out=qn[:],
                    in_=q_d[rows, h * D : (h + 1) * D].rearrange(
                        "(t p) d -> p t d", p=128
                    ),
                )
                qt = qpool.tile([128, S], BF16, tag="qt", name="qt")
                for t in range(NT):
                    nc.sync.dma_start_transpose(
                        out=qt[:, t * 128 : (t + 1) * 128], in_=qn[:, t, :]
                    )

                den = psD.tile([128, NT], F32, tag="den", name="den")
                o_ps = {}
                pgs = {}
                qoffs = {}
                for g in range(2):
                    jmax = 4 * (g + 1)
                    pg = ppool.tile([128, jmax * 512], BF16, tag=f"pg{g}", name=f"pg{g}")
                    pgs[g] = pg
                    for j in range(jmax):
                        qoff = max(512 * g, 128 * j)
                        n = 512 * (g + 1) - qoff
                        qoffs[(g, j)] = qoff
                        s_ps = psS.tile([128, 512], F32, tag="s", name="s_ps")
                        nc.tensor.matmul(
                            s_ps[:, 0:n],
                            lhsT=kt[:, j * 128 : (j + 1) * 128],
                            rhs=qt[:, qoff : qoff + n],
                            start=True,
                            stop=True,
                        )
                        nc.scalar.activation(
                            pg[:, j * 512 : j * 512 + n],
                            s_ps[:, 0:n],
                            mybir.ActivationFunctionType.Exp,
                            scale=SCALE,
                        )
                        if 128 * j >= 512 * g:
                            # diagonal block: first 128 cols hold the triangle
                            nc.vector.tensor_mul(
                                pg[:, j * 512 : j * 512 + 128],
                                pg[:, j * 512 : j * 512 + 128],
                                mask_sb[:],
                            )
                    og = psO.tile([128, 512], F32, tag="og", name="og")
                    o_ps[g] = og
                    for tq in range(4 * g, 4 * (g + 1)):
                        for j in range(tq + 1):
                            off = j * 512 + (128 * tq - qoffs[(g, j)])
                            lhs = pg[:, off : off + 128]
                            nc.tensor.matmul(
                                og[:, (tq - 4 * g) * 128 : (tq - 4 * g + 1) * 128],
                                lhsT=lhs,
                                rhs=vn[:, j, :],
                                start=(j == 0),
                                stop=(j == tq),
                            )
                            nc.tensor.matmul(
                                den[:, tq : tq + 1],
                                lhsT=lhs,
                                rhs=ones_sb[:],
                                start=(j == 0),
                                stop=(j == tq),
                            )
                recip = rpool.tile([128, NT], F32, tag="recip", name="recip")
                nc.vector.reciprocal(recip[:], den[:])
                for g in range(2):
                    o_sb = opool.tile([128, 512], F32, tag="osb", name="o_sb")
                    rsl = recip[:, 4 * g : 4 * g + 4]
                    rb = bass.AP(
                        rsl.tensor, rsl.offset, [rsl.ap[0], rsl.ap[1], [0, 128]]
                    )
                    nc.vector.tensor_mul(
                        o_sb.rearrange("p (t d) -> p t d", t=4),
                        o_ps[g].rearrange("p (t d) -> p t d", t=4),
                        rb,
                    )
                    nc.sync.dma_start(
                        out=o_d[
                            b * S + 512 * g : b * S + 512 * (g + 1),
                            h * D : (h + 1) * D,
                        ].rearrange("(t p) d -> p t d", p=128),
                        in_=o_sb.rearrange("p (t d) -> p t d", t=4),
                    )
    nc.compile()
    return nc


def _consts():
    trimask = np.triu(np.ones((128, 128))).astype(ml_dtypes.bfloat16)
    onecol = np.ones((128, 1), dtype=ml_dtypes.bfloat16)
    return trimask, onecol


def _shard_inputs(q, k, v):
    trimask, onecol = _consts()
    q = np.ascontiguousarray(np.asarray(q, dtype=np.float32))
    k = np.ascontiguousarray(np.asarray(k, dtype=np.float32))
    v = np.ascontiguousarray(np.asarray(v, dtype=np.float32))
    in_maps = []
    for c in range(NC):
        in_maps.append(
            {
                "q": np.ascontiguousarray(q[:, 512 * c : 512 * (c + 1)]),
                "k": np.ascontiguousarray(k[:, 128 * c : 128 * (c + 1)]),
                "v": np.ascontiguousarray(v[:, 128 * c : 128 * (c + 1)]),
                "trimask": trimask,
                "onecol": onecol,
            }
        )
    return in_maps


def kernel(q, k, v, cu_seqlens_q, cu_seqlens_k, _trace=False, _trace_kwargs=None):
    if "nc" not in _CACHE:
        _CACHE["nc"] = _build_nc()
    nc = _CACHE["nc"]
    in_maps = _shard_inputs(q, k, v)
    res = run_bass_kernel_spmd(
        nc, in_maps, core_ids=list(range(NC)), trace=_trace,
        **(_trace_kwargs or {}),
    )
    _CACHE["last_result"] = res
    o = np.concatenate([res.results[c]["o"] for c in range(NC)], axis=1)
    return o.astype(np.float32, copy=False)


# revision 17
# speedup vs baseline: 35.2455x; 35.2455x over previous
"""Causal varlen GQA flash attention (prefill) on 8 TRN2 NeuronCores.

Problem shape (hardcoded): B=8 sequences x S=1024 tokens, 32 q heads /
8 kv heads (GQA group 4), head_dim 128, fp32 in/out, causal.

Sharding: tensor-parallel over kv heads. Core c owns kv head c and its
4 query heads: q cols [512c, 512c+512), k/v cols [128c, 128c+128),
output cols [512c, 512c+512). No collectives; host concatenates.

Per-core kernel (bf16 matmuls, fp32 PSUM accumulation):
  S^T[k,q] = (K^T block).T @ Q^T       PE, causally trimmed N, blocks
                                       packed into shared PSUM regions
  P^T      = exp(scale * S^T)          ScalarE, one op per packed region
  diagonal blocks masked (triangular)  DVE
  O[q,d]  += (P^T block).T @ V block   PE (P^T stationary, V moving)
  den[q]  += (P^T block).T @ ones      PE (same stationary, N=1)
  out      = O * (1/den)               DVE reciprocal + broadcast mul
"""

import numpy as np
import ml_dtypes
from contextlib import ExitStack

import concourse.bacc as bacc
import concourse.bass as bass
import concourse.mybir as mybir
import concourse.tile as tile
from concourse.bass_utils import run_bass_kernel_spmd

B = 8
S = 1024
D = 128
GH = 4            # q heads per core
NT = S // 128     # 128-token tiles per sequence
NC = 8            # cores
SCALE = 1.0 / float(np.sqrt(D))
F32 = mybir.dt.float32
BF16 = mybir.dt.bfloat16

# Packed S^T regions per q-group: list of (tag, [(j, off_in_region, N)]).
# Within a region every matmul output stays inside one 2KB PSUM bank and
# the valid (causally trimmed) columns are contiguous, so one exp covers
# the whole region with zero waste.
REGIONS = {
    0: [
        ("s2", [(0, 0, 512), (1, 512, 384), (3, 896, 128)]),
        ("s1", [(2, 0, 256)]),
    ],
    1: [
        ("s2", [(0, 0, 512), (1, 512, 512)]),
        ("s2", [(2, 0, 512), (3, 512, 512)]),
        ("s2", [(4, 0, 512), (5, 512, 384), (7, 896, 128)]),
        ("s1", [(6, 0, 256)]),
    ],
}
PG_SIZE = {0: 1280, 1: 3328}

_CACHE: dict = {}


def _build_nc(b_count=B, h_count=GH, rep_count=1):
    nc = bacc.Bacc("TRN2", target_bir_lowering=False, debug=False)
    q_d = nc.dram_tensor("q", [B * S, GH * D], F32, kind="ExternalInput")
    k_d = nc.dram_tensor("k", [B * S, D], F32, kind="ExternalInput")
    v_d = nc.dram_tensor("v", [B * S, D], F32, kind="ExternalInput")
    m_d = nc.dram_tensor("trimask", [128, 128], BF16, kind="ExternalInput")
    one_d = nc.dram_tensor("onecol", [128, 1], BF16, kind="ExternalInput")
    o_d = nc.dram_tensor("o", [B * S, GH * D], F32, kind="ExternalOutput")

    with tile.TileContext(nc) as tc, ExitStack() as ctx:
        cpool = ctx.enter_context(tc.tile_pool(name="const", bufs=1))
        kvpool = ctx.enter_context(tc.tile_pool(name="kv", bufs=2))
        qpool = ctx.enter_context(tc.tile_pool(name="qp", bufs=2))
        ppool = ctx.enter_context(tc.tile_pool(name="pp", bufs=2))
        opool = ctx.enter_context(tc.tile_pool(name="op", bufs=2))
        rpool = ctx.enter_context(tc.tile_pool(name="rp", bufs=2))
        psS = ctx.enter_context(tc.tile_pool(name="psS", bufs=2, space="PSUM"))
        psO = ctx.enter_context(tc.tile_pool(name="psO", bufs=2, space="PSUM"))
        psD = ctx.enter_context(tc.tile_pool(name="psD", bufs=1, space="PSUM"))

        mask_sb = cpool.tile([128, 128], BF16, name="mask_sb")
        nc.sync.dma_start(out=mask_sb[:], in_=m_d[:])
        ones_sb = cpool.tile([128, 1], BF16, name="ones_sb")
        nc.sync.dma_start(out=ones_sb[:], in_=one_d[:])

        def emit_kv_load(b):
            # K/V ride HWDGE as fp32 and convert on DVE — keeps the big
            # casting loads off the SWDGE descriptor ring (Q saturates it)
            rows = slice(b * S, (b + 1) * S)
            knf = kvpool.tile([128, NT, 128], F32, tag="knf", name="knf")
            nc.sync.dma_start(
                out=knf[:], in_=k_d[rows, :].rearrange("(t p) d -> p t d", p=128)
            )
            kn = kvpool.tile([128, NT, 128], BF16, tag="kn", name="kn")
            nc.vector.tensor_copy(kn[:], knf[:])
            vnf = kvpool.tile([128, NT, 128], F32, tag="vnf", name="vnf")
            nc.sync.dma_start(
                out=vnf[:], in_=v_d[rows, :].rearrange("(t p) d -> p t d", p=128)
            )
            vn = kvpool.tile([128, NT, 128], BF16, tag="vn", name="vn")
            nc.vector.tensor_copy(vn[:], vnf[:])
            kt = kvpool.tile([128, S], BF16, tag="kt", name="kt")
            nc.sync.dma_start_transpose(
                out=kt.rearrange("d (t p) -> d t p", t=NT),
                in_=kn.rearrange("p t d -> p (t d)"),
            )
            return kt, vn

        def emit_q_load(b):
            rows = slice(b * S, (b + 1) * S)
            qn = qpool.tile([128, NT, GH * 128], BF16, tag="qn", name="qn")
            nc.gpsimd.dma_start(
                out=qn[:],
                in_=q_d[rows, :].rearrange("(t p) hd -> p t hd", p=128),
            )
            qta = qpool.tile([128, NT * GH * 128], BF16, tag="qt", name="qta")
            nc.sync.dma_start_transpose(
                out=qta.rearrange("d (th p) -> d th p", p=128),
                in_=qn.rearrange("p t hd -> p (t hd)"),
            )
            # [d, t, h, p]: head h's q-tile t lives at free (t*GH+h)*128+p
            return qta.rearrange("d (t h p) -> d t h p", h=GH, p=128)

        # fast-start staging for the very first pair: a small head-0-only
        # load beats waiting for the full 4-head load + transpose
        qn0 = qpool.tile([128, NT, 128], BF16, tag="qn0", name="qn0", bufs=1)
        nc.gpsimd.dma_start(
            out=qn0[:], in_=q_d[0:S, 0:D].rearrange("(t p) d -> p t d", p=128)
        )
        qt0 = qpool.tile([128, S], BF16, tag="qt0", name="qt0", bufs=1)
        nc.sync.dma_start_transpose(
            out=qt0.rearrange("d (t p) -> d t p", p=128),
            in_=qn0.rearrange("p t d -> p (t d)"),
        )
        qt0_view = qt0.rearrange("d (t p) -> d t p", p=128)

        kv_tiles = {0: emit_kv_load(0)}
        q_tiles = {0: emit_q_load(0)}
        for rep in range(rep_count):
          for b in range(b_count):
            kt, vn = kv_tiles.pop(b) if (b in kv_tiles) else (None, None)
            if kt is None:
                kt, vn = emit_kv_load(b)
            qt4 = q_tiles.pop(b) if (b in q_tiles) else emit_q_load(b)
            for h in range(h_count):
                qth = qt0_view if (rep == 0 and b == 0 and h == 0) else qt4[:, :, h, :]
                if h == 0 and b + 1 < b_count:
                    q_tiles[b + 1] = emit_q_load(b + 1)
                    kv_tiles[b + 1] = emit_kv_load(b + 1)

                den = psD.tile([128, NT], F32, tag="den", name="den")
                o_ps = {}
                pgs = {}
                pgoff = {}   # (g, j) -> offset in pg
                qoff = {}    # (g, j) -> absolute first valid q column
                for g in range(2):
                    pg = ppool.tile(
                        [128, PG_SIZE[g]], BF16, tag=f"pg{g}", name=f"pg{g}"
                    )
                    pgs[g] = pg
                    base = 0
                    for tag, blocks in REGIONS[g]:
                        tot = sum(n for _, _, n in blocks)
                        width = 1024 if tag == "s2" else 256
                        s_t = psS.tile(
                            [128, width], F32, tag=tag, name="s_t",
                            bufs=(2 if tag == "s2" else 1),
                        )
                        for j, off, n in blocks:
                            qo = 512 * (g + 1) - n
                            qoff[(g, j)] = qo
                            pgoff[(g, j)] = base + off
                            nc.tensor.matmul(
                                s_t[:, off : off + n],
                                lhsT=kt[:, j * 128 : (j + 1) * 128],
                                rhs=qth[:, qo // 128 : (qo + n) // 128, :],
                                start=True,
                                stop=True,
                            )
                        nc.scalar.activation(
                            pg[:, base : base + tot],
                            s_t[:, 0:tot],
                            mybir.ActivationFunctionType.Exp,
                            scale=SCALE,
                        )
                        for j, off, n in blocks:
                            if 128 * j >= 512 * g:
                                # diagonal block: first 128 cols are the triangle
                                nc.vector.tensor_mul(
                                    pg[:, base + off : base + off + 128],
                                    pg[:, base + off : base + off + 128],
                                    mask_sb[:],
                                )
                        base += tot
                    og = psO.tile([128, 512], F32, tag="og", name="og")
                    o_ps[g] = og
                    for tq in range(4 * g, 4 * (g + 1)):
                        for j in range(tq + 1):
                            off = pgoff[(g, j)] + (128 * tq - qoff[(g, j)])
                            lhs = pg[:, off : off + 128]
                            nc.tensor.matmul(
                                og[:, (tq - 4 * g) * 128 : (tq - 4 * g + 1) * 128],
                                lhsT=lhs,
                                rhs=vn[:, j, :],
                                start=(j == 0),
                                stop=(j == tq),
                            )
                            nc.tensor.matmul(
                                den[:, tq : tq + 1],
                                lhsT=lhs,
                                rhs=ones_sb[:],
                                start=(j == 0),
                                stop=(j == tq),
                            )
                recip = rpool.tile([128, NT], F32, tag="recip", name="recip")
                nc.vector.reciprocal(recip[:], den[:])
                for g in range(2):
                    o_sb = opool.tile([128, 512], F32, tag="osb", name="o_sb", bufs=4)
                    rsl = recip[:, 4 * g : 4 * g + 4]
                    rb = bass.AP(
                        rsl.tensor, rsl.offset, [rsl.ap[0], rsl.ap[1], [0, 128]]
                    )
                    nc.vector.tensor_mul(
                        o_sb.rearrange("p (t d) -> p t d", t=4),
                        o_ps[g].rearrange("p (t d) -> p t d", t=4),
                        rb,
                    )
                    nc.sync.dma_start(
                        out=o_d[
                            b * S + 512 * g : b * S + 512 * (g + 1),
                            h * D : (h + 1) * D,
                        ].rearrange("(t p) d -> p t d", p=128),
                        in_=o_sb.rearrange("p (t d) -> p t d", t=4),
                    )
    nc.compile()
    return nc


def _consts():
    trimask = np.triu(np.ones((128, 128))).astype(ml_dtypes.bfloat16)
    onecol = np.ones((128, 1), dtype=ml_dtypes.bfloat16)
    return trimask, onecol


def _shard_inputs(q, k, v):
    trimask, onecol = _consts()
    q = np.ascontiguousarray(np.asarray(q, dtype=np.float32))
    k = np.ascontiguousarray(np.asarray(k, dtype=np.float32))
    v = np.ascontiguousarray(np.asarray(v, dtype=np.float32))
    in_maps = []
    for c in range(NC):
        in_maps.append(
            {
                "q": np.ascontiguousarray(q[:, 512 * c : 512 * (c + 1)]),
                "k": np.ascontiguousarray(k[:, 128 * c : 128 * (c + 1)]),
                "v": np.ascontiguousarray(v[:, 128 * c : 128 * (c + 1)]),
                "trimask": trimask,
                "onecol": onecol,
            }
        )
    return in_maps


def kernel(q, k, v, cu_seqlens_q, cu_seqlens_k, _trace=False, _trace_kwargs=None):
    if "nc" not in _CACHE:
        _CACHE["nc"] = _build_nc()
    nc = _CACHE["nc"]
    in_maps = _shard_inputs(q, k, v)
    res = run_bass_kernel_spmd(
        nc, in_maps, core_ids=list(range(NC)), trace=_trace,
        **(_trace_kwargs or {}),
    )
    _CACHE["last_result"] = res
    o = np.concatenate([res.results[c]["o"] for c in range(NC)], axis=1)
    return o.astype(np.float32, copy=False)


# revision 18
# speedup vs baseline: 45.2239x; 1.2831x over previous
"""Causal varlen GQA flash attention (prefill) on 8 TRN2 NeuronCores.

Problem shape (hardcoded): B=8 sequences x S=1024 tokens, 32 q heads /
8 kv heads (GQA group 4), head_dim 128, fp32 in/out, causal.

Sharding: tensor-parallel over kv heads. Core c owns kv head c and its
4 query heads: q cols [512c, 512c+512), k/v cols [128c, 128c+128),
output cols [512c, 512c+512). No collectives; host concatenates.

Per-core kernel (bf16 matmuls, fp32 PSUM accumulation):
  S^T[k,q] = (K^T block).T @ Q^T       PE, causally trimmed N, blocks
                                       packed into shared PSUM regions
  P^T      = exp(scale * S^T)          ScalarE, one op per packed region
  diagonal blocks masked (triangular)  DVE
  O[q,d]  += (P^T block).T @ V block   PE (P^T stationary, V moving)
  den[q]  += (P^T block).T @ ones      PE (same stationary, N=1)
  out      = O * (1/den)               DVE reciprocal + broadcast mul
"""

import numpy as np
import ml_dtypes
from contextlib import ExitStack

import concourse.bacc as bacc
import concourse.bass as bass
import concourse.mybir as mybir
import concourse.tile as tile
from concourse.bass_utils import run_bass_kernel_spmd

B = 8
S = 1024
D = 128
GH = 4            # q heads per core
NT = S // 128     # 128-token tiles per sequence
NC = 8            # cores
SCALE = 1.0 / float(np.sqrt(D))
F32 = mybir.dt.float32
BF16 = mybir.dt.bfloat16

# Packed S^T regions per q-group: list of (tag, [(j, off_in_region, N)]).
# Within a region every matmul output stays inside one 2KB PSUM bank and
# the valid (causally trimmed) columns are contiguous, so one exp covers
# the whole region with zero waste.
REGIONS = {
    0: [
        ("sA", [(0, 0, 512), (1, 512, 384), (3, 896, 128), (2, 1024, 256)]),
    ],
    1: [
        ("sB", [(0, 0, 512), (1, 512, 512)]),
        ("sA", [(2, 0, 512), (3, 512, 512), (6, 1024, 256)]),
        ("sB", [(4, 0, 512), (5, 512, 384), (7, 896, 128)]),
    ],
}
REGION_WIDTH = {"sA": 1280, "sB": 1024}
PG_SIZE = {0: 1280, 1: 3328}

_CACHE: dict = {}


def _build_nc(b_count=B, h_count=GH, rep_count=1):
    nc = bacc.Bacc("TRN2", target_bir_lowering=False, debug=False)
    q_d = nc.dram_tensor("q", [B * S, GH * D], F32, kind="ExternalInput")
    k_d = nc.dram_tensor("k", [B * S, D], F32, kind="ExternalInput")
    v_d = nc.dram_tensor("v", [B * S, D], F32, kind="ExternalInput")
    m_d = nc.dram_tensor("trimask", [128, 128], BF16, kind="ExternalInput")
    one_d = nc.dram_tensor("onecol", [128, 1], BF16, kind="ExternalInput")
    o_d = nc.dram_tensor("o", [B * S, GH * D], F32, kind="ExternalOutput")

    with tile.TileContext(nc) as tc, ExitStack() as ctx:
        cpool = ctx.enter_context(tc.tile_pool(name="const", bufs=1))
        kvpool = ctx.enter_context(tc.tile_pool(name="kv", bufs=2))
        qpool = ctx.enter_context(tc.tile_pool(name="qp", bufs=2))
        ppool = ctx.enter_context(tc.tile_pool(name="pp", bufs=2))
        opool = ctx.enter_context(tc.tile_pool(name="op", bufs=2))
        rpool = ctx.enter_context(tc.tile_pool(name="rp", bufs=2))
        psS = ctx.enter_context(tc.tile_pool(name="psS", bufs=2, space="PSUM"))
        psO = ctx.enter_context(tc.tile_pool(name="psO", bufs=2, space="PSUM"))
        psD = ctx.enter_context(tc.tile_pool(name="psD", bufs=1, space="PSUM"))

        mask_sb = cpool.tile([128, 128], BF16, name="mask_sb")
        nc.sync.dma_start(out=mask_sb[:], in_=m_d[:])
        ones_sb = cpool.tile([128, 1], BF16, name="ones_sb")
        nc.sync.dma_start(out=ones_sb[:], in_=one_d[:])

        def emit_kv_load(b):
            # K/V ride HWDGE as fp32 and convert on DVE — keeps the big
            # casting loads off the SWDGE descriptor ring (Q saturates it)
            rows = slice(b * S, (b + 1) * S)
            knf = kvpool.tile([128, NT, 128], F32, tag="knf", name="knf")
            nc.sync.dma_start(
                out=knf[:], in_=k_d[rows, :].rearrange("(t p) d -> p t d", p=128)
            )
            kn = kvpool.tile([128, NT, 128], BF16, tag="kn", name="kn")
            nc.vector.tensor_copy(kn[:], knf[:])
            vnf = kvpool.tile([128, NT, 128], F32, tag="vnf", name="vnf")
            nc.sync.dma_start(
                out=vnf[:], in_=v_d[rows, :].rearrange("(t p) d -> p t d", p=128)
            )
            vn = kvpool.tile([128, NT, 128], BF16, tag="vn", name="vn")
            nc.vector.tensor_copy(vn[:], vnf[:])
            kt = kvpool.tile([128, S], BF16, tag="kt", name="kt")
            nc.sync.dma_start_transpose(
                out=kt.rearrange("d (t p) -> d t p", t=NT),
                in_=kn.rearrange("p t d -> p (t d)"),
            )
            return kt, vn

        def emit_q_load(b):
            rows = slice(b * S, (b + 1) * S)
            qn = qpool.tile([128, NT, GH * 128], BF16, tag="qn", name="qn")
            nc.gpsimd.dma_start(
                out=qn[:],
                in_=q_d[rows, :].rearrange("(t p) hd -> p t hd", p=128),
            )
            qta = qpool.tile([128, NT * GH * 128], BF16, tag="qt", name="qta")
            nc.sync.dma_start_transpose(
                out=qta.rearrange("d (th p) -> d th p", p=128),
                in_=qn.rearrange("p t hd -> p (t hd)"),
            )
            # [d, t, h, p]: head h's q-tile t lives at free (t*GH+h)*128+p
            return qta.rearrange("d (t h p) -> d t h p", h=GH, p=128)

        # fast-start staging for the very first pair: a small head-0-only
        # load beats waiting for the full 4-head load + transpose
        qn0 = qpool.tile([128, NT, 128], BF16, tag="qn0", name="qn0", bufs=1)
        nc.gpsimd.dma_start(
            out=qn0[:], in_=q_d[0:S, 0:D].rearrange("(t p) d -> p t d", p=128)
        )
        qt0 = qpool.tile([128, S], BF16, tag="qt0", name="qt0", bufs=1)
        nc.sync.dma_start_transpose(
            out=qt0.rearrange("d (t p) -> d t p", p=128),
            in_=qn0.rearrange("p t d -> p (t d)"),
        )
        qt0_view = qt0.rearrange("d (t p) -> d t p", p=128)

        kv_tiles = {0: emit_kv_load(0)}
        q_tiles = {0: emit_q_load(0)}
        for rep in range(rep_count):
          for b in range(b_count):
            kt, vn = kv_tiles.pop(b) if (b in kv_tiles) else (None, None)
            if kt is None:
                kt, vn = emit_kv_load(b)
            qt4 = q_tiles.pop(b) if (b in q_tiles) else emit_q_load(b)
            for h in range(h_count):
                qth = qt0_view if (rep == 0 and b == 0 and h == 0) else qt4[:, :, h, :]
                if h == 0 and (b + 1 < b_count or rep + 1 < rep_count):
                    nb = (b + 1) % b_count
                    q_tiles[nb] = emit_q_load(nb)
                    kv_tiles[nb] = emit_kv_load(nb)

                den = psD.tile([128, NT], F32, tag="den", name="den")
                o_ps = {}
                pgs = {}
                pgoff = {}   # (g, j) -> offset in pg
                qoff = {}    # (g, j) -> absolute first valid q column
                for g in range(2):
                    pg = ppool.tile(
                        [128, PG_SIZE[g]], BF16, tag=f"pg{g}", name=f"pg{g}"
                    )
                    pgs[g] = pg
                    base = 0
                    for tag, blocks in REGIONS[g]:
                        tot = sum(n for _, _, n in blocks)
                        s_t = psS.tile(
                            [128, REGION_WIDTH[tag]], F32, tag=tag, name="s_t",
                            bufs=1,
                        )
                        for j, off, n in blocks:
                            qo = 512 * (g + 1) - n
                            qoff[(g, j)] = qo
                            pgoff[(g, j)] = base + off
                            nc.tensor.matmul(
                                s_t[:, off : off + n],
                                lhsT=kt[:, j * 128 : (j + 1) * 128],
                                rhs=qth[:, qo // 128 : (qo + n) // 128, :],
                                start=True,
                                stop=True,
                            )
                        nc.scalar.activation(
                            pg[:, base : base + tot],
                            s_t[:, 0:tot],
                            mybir.ActivationFunctionType.Exp,
                            scale=SCALE,
                        )
                        for j, off, n in blocks:
                            if 128 * j >= 512 * g:
                                # diagonal block: first 128 cols are the triangle
                                nc.vector.tensor_mul(
                                    pg[:, base + off : base + off + 128],
                                    pg[:, base + off : base + off + 128],
                                    mask_sb[:],
                                )
                        base += tot
                    og = psO.tile([128, 512], F32, tag="og", name="og")
                    o_ps[g] = og
                    for tq in range(4 * g, 4 * (g + 1)):
                        for j in range(tq + 1):
                            off = pgoff[(g, j)] + (128 * tq - qoff[(g, j)])
                            lhs = pg[:, off : off + 128]
                            nc.tensor.matmul(
                                og[:, (tq - 4 * g) * 128 : (tq - 4 * g + 1) * 128],
                                lhsT=lhs,
                                rhs=vn[:, j, :],
                                start=(j == 0),
                                stop=(j == tq),
                            )
                            nc.tensor.matmul(
                                den[:, tq : tq + 1],
                                lhsT=lhs,
                                rhs=ones_sb[:],
                                start=(j == 0),
                                stop=(j == tq),
                            )
                recip = rpool.tile([128, NT], F32, tag="recip", name="recip")
                nc.vector.reciprocal(recip[:], den[:])
                for g in range(2):
                    o_sb = opool.tile([128, 512], F32, tag="osb", name="o_sb", bufs=4)
                    rsl = recip[:, 4 * g : 4 * g + 4]
                    rb = bass.AP(
                        rsl.tensor, rsl.offset, [rsl.ap[0], rsl.ap[1], [0, 128]]
                    )
                    nc.vector.tensor_mul(
                        o_sb.rearrange("p (t d) -> p t d", t=4),
                        o_ps[g].rearrange("p (t d) -> p t d", t=4),
                        rb,
                    )
                    nc.sync.dma_start(
                        out=o_d[
                            b * S + 512 * g : b * S + 512 * (g + 1),
                            h * D : (h + 1) * D,
                        ].rearrange("(t p) d -> p t d", p=128),
                        in_=o_sb.rearrange("p (t d) -> p t d", t=4),
                    )
    nc.compile()
    return nc


def _consts():
    trimask = np.triu(np.ones((128, 128))).astype(ml_dtypes.bfloat16)
    onecol = np.ones((128, 1), dtype=ml_dtypes.bfloat16)
    return trimask, onecol


def _shard_inputs(q, k, v):
    trimask, onecol = _consts()
    q = np.ascontiguousarray(np.asarray(q, dtype=np.float32))
    k = np.ascontiguousarray(np.asarray(k, dtype=np.float32))
    v = np.ascontiguousarray(np.asarray(v, dtype=np.float32))
    in_maps = []
    for c in range(NC):
        in_maps.append(
            {
                "q": np.ascontiguousarray(q[:, 512 * c : 512 * (c + 1)]),
                "k": np.ascontiguousarray(k[:, 128 * c : 128 * (c + 1)]),
                "v": np.ascontiguousarray(v[:, 128 * c : 128 * (c + 1)]),
                "trimask": trimask,
                "onecol": onecol,
            }
        )
    return in_maps


def kernel(q, k, v, cu_seqlens_q, cu_seqlens_k, _trace=False, _trace_kwargs=None):
    if "nc" not in _CACHE:
        _CACHE["nc"] = _build_nc()
    nc = _CACHE["nc"]
    in_maps = _shard_inputs(q, k, v)
    res = run_bass_kernel_spmd(
        nc, in_maps, core_ids=list(range(NC)), trace=_trace,
        **(_trace_kwargs or {}),
    )
    _CACHE["last_result"] = res
    o = np.concatenate([res.results[c]["o"] for c in range(NC)], axis=1)
    return o.astype(np.float32, copy=False)
